# revision 7
# baseline (speedup 1.0000x reference)
"""Chamfer distance kernel for Trainium2 (8 NeuronCores).

Problem: input1 [4,8192,3], input2 [4,8192,3] f32.
  d2[b,n,m] = ||x_bn - y_bm||^2 (clamped at 0)
  out = mean_n(min_m d2) + mean_m(min_n d2)   (scalar f32)

Sharding: 8 cores = 4 batches x 2 halves of N. Each core computes its
4096x8192 block of the distance matrix twice (normal + transposed
orientation by swapping matmul operands) so that BOTH chamfer directions
become row-min reductions.

Distance tiles are produced by a single K=5 matmul:
  d2[n,m] = [x2_n, 1, x_n0, x_n1, x_n2] . [1, y2_m, -2y_m0, -2y_m1, -2y_m2]
with float32r inputs (full-rate PE). The DVE min-accumulates 2048-wide
PSUM supertiles; the last accumulate of each row-group is a fused
tensor_tensor_reduce that also emits the row min. Host merges the two
N-half partial mins for dist2, clamps at 0 and takes the means.
"""

import os
import sys

import numpy as np

for _p in ("/opt/trn_rl_repo", "/root/.axon_site/_ro/trn_rl_repo"):
    if os.path.isdir(_p) and _p not in sys.path:
        sys.path.insert(0, _p)
        break

import concourse.bass as bass
import concourse.tile as tile
from concourse import bacc
from concourse import mybir
from concourse.bass_utils import run_bass_kernel_spmd

B, N, M, D = 4, 8192, 8192, 3
NCORES = 8
HALF = N // 2
BIG = 3.0e38

_prog_cache: dict = {}


def build_program(n_rows: int = HALF, m_cols: int = M) -> bass.Bass:
    """One-core program: inputs a_s/a_m [5,n_rows], b_s/b_m [5,m_cols]
    (stationary/moving augmented forms of the x half and y), outputs
    out1 [n_rows] = min_m d2 rows, out2 [m_cols] = min over this n-half."""
    f32 = mybir.dt.float32
    f32r = mybir.dt.float32r
    mn = mybir.AluOpType.min

    NT1 = n_rows // 128
    NT2 = m_cols // 128
    S1 = min(2048, m_cols)
    SB1 = m_cols // S1
    S2 = min(2048, n_rows)
    SB2 = n_rows // S2
    assert n_rows % 128 == 0 and m_cols % 128 == 0
    assert m_cols % S1 == 0 and S1 % 512 == 0
    assert n_rows % S2 == 0 and S2 % 512 == 0

    nc = bacc.Bacc()
    # single packed input [a_s | a_m | b_s | b_m] along the free dim so the
    # kernel needs exactly ONE input DMA (self-loading f32r matmuls can only
    # carry a single sync wait — walrus S3_LW limit)
    W = 2 * (n_rows + m_cols)
    aug = nc.declare_dram_parameter("aug", [5, W], f32r, isOutput=False)
    out1 = nc.declare_dram_parameter("out1", [n_rows], f32, isOutput=True)
    out2 = nc.declare_dram_parameter("out2", [m_cols], f32, isOutput=True)

    with tile.TileContext(nc) as tc:
        with (
            tc.tile_pool(name="consts", bufs=1) as consts,
            tc.tile_pool(name="accp", bufs=4) as accp,
            tc.tile_pool(name="psump", bufs=2, space="PSUM") as psump,
        ):
            aug_t = consts.tile([5, W], f32r)
            nc.sync.dma_start(out=aug_t, in_=aug[:, :])
            as_t = aug_t[:, 0:n_rows]
            am_t = aug_t[:, n_rows : 2 * n_rows]
            bs_t = aug_t[:, 2 * n_rows : 2 * n_rows + m_cols]
            bm_t = aug_t[:, 2 * n_rows + m_cols : W]
            R1 = consts.tile([128, NT1], f32)
            R2 = consts.tile([128, NT2], f32)

            def one_pass(NT, SB, S, stat, mov, R):
                Q = S // 512
                for i in range(NT):
                    acc = accp.tile([128, S], f32, tag="acc")
                    for jj in range(SB):
                        ps = psump.tile([128, S], f32, tag="ps")
                        for q in range(Q):
                            col = jj * S + q * 512
                            nc.tensor.matmul(
                                ps[:, q * 512 : (q + 1) * 512],
                                lhsT=stat[:, i * 128 : (i + 1) * 128],
                                rhs=mov[:, col : col + 512],
                                start=True,
                                stop=True,
                            )
                        if SB == 1:
                            # single block: copy-min then explicit reduce
                            nc.vector.tensor_scalar_min(out=acc, in0=ps, scalar1=BIG)
                            nc.vector.tensor_reduce(
                                out=R[:, i : i + 1],
                                in_=acc,
                                axis=mybir.AxisListType.X,
                                op=mn,
                            )
                        elif jj == 0:
                            nc.vector.tensor_scalar_min(out=acc, in0=ps, scalar1=BIG)
                        elif jj < SB - 1:
                            nc.vector.tensor_tensor(out=acc, in0=ps, in1=acc, op=mn)
                        else:
                            # last block: elementwise min, then row-min reduce
                            # (tensor_tensor_reduce crashes at runtime on this
                            # toolchain, so reduce explicitly)
                            nc.vector.tensor_tensor(out=acc, in0=ps, in1=acc, op=mn)
                            nc.vector.tensor_reduce(
                                out=R[:, i : i + 1],
                                in_=acc,
                                axis=mybir.AxisListType.X,
                                op=mn,
                            )

            one_pass(NT1, SB1, S1, as_t, bm_t, R1)  # dist1 rows
            one_pass(NT2, SB2, S2, bs_t, am_t, R2)  # dist2 rows (this n-half)

            nc.sync.dma_start(
                out=out1[:].rearrange("(i p) -> p i", p=128), in_=R1
            )
            nc.sync.dma_start(
                out=out2[:].rearrange("(i p) -> p i", p=128), in_=R2
            )

    nc.finalize()
    return nc


def _get_program(n_rows: int, m_cols: int) -> bass.Bass:
    key = (n_rows, m_cols)
    if key not in _prog_cache:
        _prog_cache[key] = build_program(n_rows, m_cols)
    return _prog_cache[key]


def _aug(pts: np.ndarray):
    """pts [n,3] -> (stationary [5,n], moving [5,n]) augmented forms."""
    pts = np.asarray(pts, np.float32)
    sq = (pts * pts).sum(-1)
    ones = np.ones_like(sq)
    stat = np.ascontiguousarray(
        np.stack([sq, ones, pts[:, 0], pts[:, 1], pts[:, 2]]), dtype=np.float32
    )
    movg = np.ascontiguousarray(
        np.stack([ones, sq, -2.0 * pts[:, 0], -2.0 * pts[:, 1], -2.0 * pts[:, 2]]),
        dtype=np.float32,
    )
    return stat, movg


def pack_aug(x: np.ndarray, y: np.ndarray) -> np.ndarray:
    a_s, a_m = _aug(x)
    b_s, b_m = _aug(y)
    return np.ascontiguousarray(
        np.concatenate([a_s, a_m, b_s, b_m], axis=1), dtype=np.float32
    )


def make_in_maps(input1: np.ndarray, input2: np.ndarray):
    in_maps = []
    for c in range(NCORES):
        b, h = divmod(c, 2)
        x = input1[b, h * HALF : (h + 1) * HALF]
        y = input2[b]
        in_maps.append({"aug": pack_aug(x, y)})
    return in_maps


def combine(results) -> np.ndarray:
    d1 = np.zeros((B, N), np.float32)
    d2 = np.full((B, M), np.float32(BIG), np.float32)
    for c in range(NCORES):
        b, h = divmod(c, 2)
        d1[b, h * HALF : (h + 1) * HALF] = results[c]["out1"]
        d2[b] = np.minimum(d2[b], results[c]["out2"])
    d1 = np.maximum(d1, 0.0)
    d2 = np.maximum(d2, 0.0)
    val = d1.mean(dtype=np.float64) + d2.mean(dtype=np.float64)
    return np.asarray(val, dtype=np.float32)


def run_on_hw(input1, input2, **kwargs):
    nc = _get_program(HALF, M)
    in_maps = make_in_maps(np.asarray(input1, np.float32), np.asarray(input2, np.float32))
    return run_bass_kernel_spmd(nc, in_maps, list(range(NCORES)), **kwargs)


def kernel(input1: np.ndarray, input2: np.ndarray) -> np.ndarray:
    res = run_on_hw(input1, input2)
    return combine(res.results)


# revision 8
# speedup vs baseline: 1.2599x; 1.2599x over previous
"""Chamfer distance kernel for Trainium2 (8 NeuronCores).

Problem: input1 [4,8192,3], input2 [4,8192,3] f32.
  d2[b,n,m] = ||x_bn - y_bm||^2 (clamped at 0)
  out = mean_n(min_m d2) + mean_m(min_n d2)   (scalar f32)

Sharding: 8 cores = 4 batches x 2 halves of N. Each core computes its
4096x8192 block of the distance matrix ONCE (fp32 matmuls for accuracy;
fp32r was measured at ~8e-3 abs error - too coarse for the min values).

Distance tiles are produced by a single K=5 fp32 matmul:
  d2[n,m] = [x2_n, 1, x_n0, x_n1, x_n2] . [1, y2_m, -2y_m0, -2y_m1, -2y_m2]
Both chamfer directions are reduced from the same PSUM supertiles:
  - dist1 (min over m): per-row-group running TT-min chain + final row reduce
  - dist2 (min over n): elementwise TT-min accumulator acc2[128, M] across
    row groups, then PE-transpose + row reduce at the end.
Host merges the two N-half partial mins for dist2, clamps at 0, means.
"""

import os
import sys

import numpy as np

for _p in ("/opt/trn_rl_repo", "/root/.axon_site/_ro/trn_rl_repo"):
    if os.path.isdir(_p) and _p not in sys.path:
        sys.path.insert(0, _p)
        break

import concourse.bass as bass
import concourse.tile as tile
from concourse import mybir, bacc
from concourse.bass_utils import run_bass_kernel_spmd
from concourse.masks import make_identity

B, N, M, D = 4, 8192, 8192, 3
NCORES = 8
HALF = N // 2
BIG = 3.0e38

_prog_cache: dict = {}


def build_program(n_rows: int = HALF, m_cols: int = M) -> bass.Bass:
    """One-core program. Input aug [5, n_rows+m_cols] = [a_s | b_m]
    (stationary aug of x-half, moving aug of y). Outputs out1 [n_rows]
    (min over m per n-row), out2 [m_cols] (min over this n-half per m)."""
    f32 = mybir.dt.float32
    mn = mybir.AluOpType.min

    S = min(2048, m_cols)
    NT = n_rows // 128
    SBm = m_cols // S
    MT = m_cols // 128
    assert n_rows % 128 == 0 and m_cols % S == 0 and S % 512 == 0

    nc = bacc.Bacc()
    W = n_rows + m_cols
    aug = nc.declare_dram_parameter("aug", [5, W], f32, isOutput=False)
    out1 = nc.declare_dram_parameter("out1", [n_rows], f32, isOutput=True)
    out2 = nc.declare_dram_parameter("out2", [m_cols], f32, isOutput=True)

    with tile.TileContext(nc) as tc:
        with (
            tc.tile_pool(name="consts", bufs=1) as consts,
            tc.tile_pool(name="accp", bufs=4) as accp,
            tc.tile_pool(name="psump", bufs=2, space="PSUM") as psump,
        ):
            aug_t = consts.tile([5, W], f32)
            nc.gpsimd.dma_start(out=aug_t, in_=aug[:, :])
            as_t = aug_t[:, 0:n_rows]
            bm_t = aug_t[:, n_rows:W]
            R1 = consts.tile([128, NT], f32)
            R2 = consts.tile([128, MT], f32)
            acc2 = consts.tile([128, m_cols], f32)
            ident = consts.tile([128, 128], f32)
            make_identity(nc, ident)

            for i in range(NT):
                acc = accp.tile([128, S], f32, tag="acc")
                for jj in range(SBm):
                    ps = psump.tile([128, S], f32, tag="ps")
                    for q in range(S // 512):
                        col = jj * S + q * 512
                        nc.tensor.matmul(
                            ps[:, q * 512 : (q + 1) * 512],
                            lhsT=as_t[:, i * 128 : (i + 1) * 128],
                            rhs=bm_t[:, col : col + 512],
                            start=True,
                            stop=True,
                        )
                    # chain1: running min over m superblocks (row mins)
                    if jj == 0:
                        nc.vector.tensor_scalar_min(out=acc, in0=ps, scalar1=BIG)
                    else:
                        nc.vector.tensor_tensor(out=acc, in0=ps, in1=acc, op=mn)
                    # chain2: elementwise min across row groups (col mins)
                    a2 = acc2[:, jj * S : (jj + 1) * S]
                    if i == 0:
                        nc.vector.tensor_scalar_min(out=a2, in0=ps, scalar1=BIG)
                    else:
                        nc.vector.tensor_tensor(out=a2, in0=ps, in1=a2, op=mn)
                nc.vector.tensor_reduce(
                    out=R1[:, i : i + 1], in_=acc, axis=mybir.AxisListType.X, op=mn
                )

            # dist2 finale: transpose acc2 128-col blocks, reduce over free dim
            for t in range(MT):
                tp = psump.tile([128, S], f32, tag="ps")
                nc.tensor.transpose(
                    tp[:, 0:128], acc2[:, t * 128 : (t + 1) * 128], ident
                )
                nc.vector.tensor_reduce(
                    out=R2[:, t : t + 1],
                    in_=tp[:, 0:128],
                    axis=mybir.AxisListType.X,
                    op=mn,
                )

            nc.gpsimd.dma_start(out=out1[:].rearrange("(i p) -> p i", p=128), in_=R1)
            nc.gpsimd.dma_start(out=out2[:].rearrange("(i p) -> p i", p=128), in_=R2)

    nc.finalize()
    return nc


def _get_program(n_rows: int, m_cols: int) -> bass.Bass:
    key = (n_rows, m_cols)
    if key not in _prog_cache:
        _prog_cache[key] = build_program(n_rows, m_cols)
    return _prog_cache[key]


def _aug(pts: np.ndarray):
    """pts [n,3] -> (stationary [5,n], moving [5,n]) augmented forms."""
    pts = np.asarray(pts, np.float32)
    sq = (pts * pts).sum(-1)
    ones = np.ones_like(sq)
    stat = np.ascontiguousarray(
        np.stack([sq, ones, pts[:, 0], pts[:, 1], pts[:, 2]]), dtype=np.float32
    )
    movg = np.ascontiguousarray(
        np.stack([ones, sq, -2.0 * pts[:, 0], -2.0 * pts[:, 1], -2.0 * pts[:, 2]]),
        dtype=np.float32,
    )
    return stat, movg


def pack_aug(x: np.ndarray, y: np.ndarray) -> np.ndarray:
    a_s, _ = _aug(x)
    _, b_m = _aug(y)
    return np.ascontiguousarray(np.concatenate([a_s, b_m], axis=1), dtype=np.float32)


def make_in_maps(input1: np.ndarray, input2: np.ndarray):
    in_maps = []
    for c in range(NCORES):
        b, h = divmod(c, 2)
        x = input1[b, h * HALF : (h + 1) * HALF]
        y = input2[b]
        in_maps.append({"aug": pack_aug(x, y)})
    return in_maps


def combine(results) -> np.ndarray:
    d1 = np.zeros((B, N), np.float32)
    d2 = np.full((B, M), np.float32(BIG), np.float32)
    for c in range(NCORES):
        b, h = divmod(c, 2)
        d1[b, h * HALF : (h + 1) * HALF] = results[c]["out1"]
        d2[b] = np.minimum(d2[b], results[c]["out2"])
    d1 = np.maximum(d1, 0.0)
    d2 = np.maximum(d2, 0.0)
    val = d1.mean(dtype=np.float64) + d2.mean(dtype=np.float64)
    return np.asarray(val, dtype=np.float32)


def run_on_hw(input1, input2, **kwargs):
    nc = _get_program(HALF, M)
    in_maps = make_in_maps(
        np.asarray(input1, np.float32), np.asarray(input2, np.float32)
    )
    return run_bass_kernel_spmd(nc, in_maps, list(range(NCORES)), **kwargs)


def kernel(input1: np.ndarray, input2: np.ndarray) -> np.ndarray:
    res = run_on_hw(input1, input2)
    return combine(res.results)
